# revision 32
# baseline (speedup 1.0000x reference)
"""Trainium2 Bass kernel for nn_BasicRecurrentEntityEncoder (v2).

Math (per paragraph b, per step t, state h [K, D]):
    g   = sigmoid(s . (h + keys))            per entity gate
    ht  = relu(h U + keys V + s W)
    upd = h + g * ht
    h'  = upd / ||upd||_2   where active (mask), else h unchanged

Device mapping (8 cores, pure data parallel, 32 paragraphs/core):
  * rows r = (paragraph, entity) = 2048 rows/core, 16 tiles of 128.
  * masked steps are identity -> host compresses each paragraph's sequence
    to its active steps; loop runs T = max active count steps with a
    per-(row, step) pad mask zeroing the gate on the tail (normalize is
    scale-invariant so no pad term is needed in delta).
  * state: upd (fp32, unnormalized), dl (per-row 1/norm), h = dl*upd
    (bf16) + hT (bf16 transposed shadow via per-group DMA transposes on
    the two HWDGE queues) feeding the PE.
  * per step, per tile i (PSUM slot i%8):
      U:  pre  = hT_i @ U        (stationary = hT tile, rhs U)      start
      kv: pre += I @ kv_i                                           accum
      g:  gdot = hT_i @ s cols   (2 cols -> ps_g)                   own group
      sw: pre += onehot_i @ sw_j[t]  (one-hot paragraph broadcast)  stop
  * gate per 4-tile group: ACT strided psum copies + SK add (DVE) ->
    ACT Sigmoid -> Pool mult by pad. Single ACT table set (sigmoid).
  * update per tile: custom DVE op upd' = upd*dl + relu(pre*gamma), then
    DVE tensor_tensor_reduce n2 = sum(upd'^2) + 1e-12.
  * delta per group: DVE tensor_scalar pow: dl = n2^-0.5.
  * h per tile on Pool: h = dl*upd (bf16); per-group DMA transpose to hT.
"""

import os
import sys

sys.path.insert(0, "/opt/trn_rl_repo")

import numpy as np
import ml_dtypes
from contextlib import ExitStack

import concourse.bass as bass
import concourse.bacc as bacc
import concourse.mybir as mybir
from concourse.tile import TileContext

BF16 = mybir.dt.bfloat16
F32 = mybir.dt.float32
AF = mybir.ActivationFunctionType
ALU = mybir.AluOpType

B, S, K, D = 256, 256, 64, 128
NCORES = 8
BL = B // NCORES  # 32 local paragraphs
NT = 16  # row tiles per core (2048 rows / 128)
NG = 4  # tile groups (4 tiles each)
NSLOT = 16  # psum slots for pre (one per tile: WAR edges stay in emission order)

# engine-placement knobs (bisectable: conservative = False everywhere)
POOL_GATE = True  # den/gam on Pool vs DVE
POOL_NEWTON = True  # newton iterations on Pool vs DVE
ACT_TTR = True  # 6 of 16 n2 reductions on ACT Square+accum vs all-DVE sq-op
ACTQ_TRANSPOSE = True  # alternate transposes across qSP/qACT vs all qSP
POOL_H_TILES = {0, 1, 2, 8, 9, 10}  # h-copies on Pool; rest on ACT
ACT_SQ_TILES = {5, 6, 7, 13, 14, 15}  # n2 via ACT Square+accum; rest DVE sq-op
BISECT = "full"  # crash-bisect variants; see bisect.py


# ------------------------------------------------------------------ custom op
def get_update_op():
    """out = in0*s0 + relu(in1*s1)   (upd*delta + relu(pre*gamma))."""
    from concourse import dve_ops as dv
    from concourse.dve_spec import Spec, Src0, Src1, C0, C1, relu, lower, _has_src1
    from concourse.dve_uop import DveOpSpec

    name = "SCALE_ADD_RELU_SCALED_ANT"
    for o in dv.OPS:
        if o.name == name:
            return o

    def _ref(in0, in1, s0, s1, imm2):
        x = in1.astype(np.float32) * s1
        x = np.nan_to_num(x, nan=0.0, posinf=np.inf, neginf=-np.inf)
        return in0.astype(np.float32) * s0 + np.maximum(x, 0.0)

    spec = Spec(body=Src0 * C0 + relu(Src1 * C1), reference=_ref)
    return _register_dve_op(name, spec)


def _register_dve_op(name, spec):
    from concourse import dve_ops as dv
    from concourse.dve_spec import lower, _has_src1
    from concourse.dve_uop import DveOpSpec

    row = max(dv._SUB_OPCODE_FOR_NAME.values()) + 1
    assert row < 0x20, "no free custom-DVE opcode rows"
    dv._SUB_OPCODE_FOR_NAME[name] = row
    shas = {}
    for ver in ("v3", "v4"):
        try:
            uops = lower(spec, ver=ver)
            shas[ver] = DveOpSpec(
                name=name, opcode=row, uops=uops, rd1_en=_has_src1(spec)
            ).sha(ver)
        except Exception:
            pass
    assert "v3" in shas, "custom op failed to lower for TRN2"
    op = dv.DveOp(name, spec, subdim=False, uops_sha=shas)
    dv.OPS.append(op)
    dv.CUSTOM_DVE_SPECS[name] = spec
    return op


def get_sq_accum_op():
    """out = in0^2 ; accum_out = s0 + sum(out)   (row sq-norm for delta)."""
    from concourse import dve_ops as dv
    from concourse.dve_spec import Spec, Src0, C0, sq
    from operator import add

    name = "SQ_ACCUM_ANT"
    for o in dv.OPS:
        if o.name == name:
            return o

    def _ref(in0, in1, s0, s1, imm2):
        b = (in0.astype(np.float32) ** 2).astype(np.float32)
        return b, s0 + b.reshape(b.shape[0], -1).sum(axis=-1, keepdims=True)

    spec = Spec(body=sq(Src0), accum=add, accum_init=C0, reference=_ref)
    return _register_dve_op(name, spec)


# ------------------------------------------------------------------ program
def build_program(T, sim=False):
    """Emit the full per-core Bass program. Returns nc."""
    op_upd = get_update_op()
    op_sq = get_sq_accum_op()
    nc = bacc.Bacc("TRN2", target_bir_lowering=False)

    # ---- I/O (packed blobs: few DMA queues -> few sem waits downstream)
    NFB = NT * 128 + T * BL + 4 * 128  # keysT | sT | U | V | W | ident
    blob_in = nc.dram_tensor("blob16", [128, NFB], BF16, kind="ExternalInput")
    oneh_in = nc.dram_tensor("oneh", [32, NT * 128], BF16, kind="ExternalInput")
    pad_in = nc.dram_tensor("pad", [128, T, NT], F32, kind="ExternalInput")
    hfin_out = nc.dram_tensor("hfin", [128, NT, 128], F32, kind="ExternalOutput")

    with ExitStack() as ctx:
        tc = ctx.enter_context(TileContext(nc))
        ec = ctx.enter_context

        # ---- persistent SBUF
        blob_sb = ec(nc.sbuf_tensor("blob_sb", [128, NFB], BF16))
        o = 0
        keysT_sb = blob_sb[:, o : o + NT * 128].rearrange(
            "p (i e) -> p i e", i=NT
        ); o += NT * 128
        sT_sb = blob_sb[:, o : o + T * BL].rearrange(
            "p (t j) -> p t j", t=T
        ); o += T * BL
        U_sb = blob_sb[:, o : o + 128]; o += 128
        V_sb = blob_sb[:, o : o + 128]; o += 128
        W_sb = blob_sb[:, o : o + 128]; o += 128
        I_sb = blob_sb[:, o : o + 128]; o += 128
        oneh_sb = ec(nc.sbuf_tensor("oneh_sb", [32, NT, 128], BF16))
        pad_sb = ec(nc.sbuf_tensor("pad_sb", [128, T, NT], F32))
        kv_sb = ec(nc.sbuf_tensor("kv_sb", [128, NT, 128], BF16))
        SK_sb = ec(nc.sbuf_tensor("SK_sb", [128, T, NT], F32))
        swj_sb = ec(nc.sbuf_tensor("swj_sb", [32, T, 128], BF16))
        h_sb = ec(nc.sbuf_tensor("h_sb", [128, NT, 128], BF16))
        hT_sb = ec(nc.sbuf_tensor("hT_sb", [128, NT, 128], BF16))
        upd_sb = ec(nc.sbuf_tensor("upd_sb", [128, NT, 128], F32))
        sq_sb = ec(nc.sbuf_tensor("sq_sb", [128, 8, 128], F32))
        dl_sb = ec(nc.sbuf_tensor("dl_sb", [128, NT], F32))
        n2_sb = ec(nc.sbuf_tensor("n2_sb", [128, NT], F32))
        glog_sb = ec(nc.sbuf_tensor("glog_sb", [128, NT], F32))
        ex_sb = ec(nc.sbuf_tensor("ex_sb", [128, NT], F32))
        den_sb = ec(nc.sbuf_tensor("den_sb", [128, NT], F32))
        gam0_sb = ec(nc.sbuf_tensor("gam0_sb", [128, NT], F32))
        gam_sb = ec(nc.sbuf_tensor("gam_sb", [128, NT], F32))
        ones_sb = ec(nc.sbuf_tensor("ones_sb", [128, NT], F32))
        c15_sb = ec(nc.sbuf_tensor("c15_sb", [128, NT], F32))
        sbias_sb = ec(nc.sbuf_tensor("sbias_sb", [128, 1], F32))
        n2h_sb = ec(nc.sbuf_tensor("n2h_sb", [128, NT], F32))
        Lf_sb = ec(nc.sbuf_tensor("Lf_sb", [128, NT], F32))
        nwt_sb = ec(nc.sbuf_tensor("nwt_sb", [128, 2, NT], F32))
        hfin_sb = ec(nc.sbuf_tensor("hfin_sb", [128, NT, 128], F32))
        # psum
        ps_pre = ec(nc.psum_tensor("ps_pre", [128, NSLOT, 128], F32))
        ps_g = ec(nc.psum_tensor("ps_g", [128, 2 * NT], F32))
        ps_aux = ec(nc.psum_tensor("ps_aux", [128, 512], F32))

        sync = nc.sync
        vec = nc.vector
        act = nc.scalar
        pool = nc.gpsimd
        pe = nc.tensor

        # ================= setup =================
        sync.dma_start(blob_sb[:], blob_in[:], max_dma_last_dim=65024)
        sync.dma_start(pad_sb[:], pad_in[:], max_dma_last_dim=65024)
        sync.dma_start(
            oneh_sb[:].rearrange("p a b -> p (a b)"), oneh_in[:],
            max_dma_last_dim=65024,
        )

        vec.memset(h_sb[:], 0)
        vec.memset(hT_sb[:], 0)
        vec.memset(upd_sb[:], 0.0)
        vec.memset(dl_sb[:], 1.0)
        vec.memset(ones_sb[:], 1.0)
        vec.memset(c15_sb[:], 1.5)
        vec.memset(sbias_sb[:], 43.9998)

        # kv = keys @ V   (natural tiles), 4 tiles per psum round
        for c in range(4):
            for k in range(4):
                i = 4 * c + k
                pe.matmul(
                    ps_aux[:, k * 128 : (k + 1) * 128],
                    lhsT=keysT_sb[:, i, :],
                    rhs=V_sb,
                    start=True,
                    stop=True,
                )
            vec.tensor_copy(
                kv_sb[:, 4 * c : 4 * (c + 1), :], ps_aux[:]
            )

        # SK[r, t] = s_{b(r), t} . keys_r
        for i in range(NT):
            for j in range(2):
                pe.matmul(
                    ps_aux[:, 0:T],
                    lhsT=keysT_sb[:, i, :],
                    rhs=sT_sb[:, :, 2 * i + j],
                    start=True,
                    stop=True,
                )
                half = slice(0, 64) if j == 0 else slice(64, 128)
                vec.tensor_copy(SK_sb[half, :, i], ps_aux[half, 0:T])

        # sw_j[j, t, e] = (s_{t,j} @ W)[e], 4 steps per psum round
        if BISECT == "noswjsetup":
            vec.memset(swj_sb[:], 0)
        else:
            for tc0 in range(0, T, 4):
                n = min(4, T - tc0)
                for k in range(n):
                    pe.matmul(
                        ps_aux[0:32, k * 128 : (k + 1) * 128],
                        lhsT=sT_sb[:, tc0 + k, :],
                        rhs=W_sb,
                        start=True,
                        stop=True,
                    )
                vec.tensor_copy(
                    swj_sb[:, tc0 : tc0 + n, :], ps_aux[0:32, 0 : n * 128]
                )

        # ================= time loop =================
        for t in range(T):
            # ---- PE: per tile [U, kv, gate, sw]; 2-col gate rides between
            # 128-col matmuls so its ldweights stays hidden.
            for i in range(NT):
                sl = i % NSLOT
                pe.matmul(
                    ps_pre[:, sl, :],
                    lhsT=hT_sb[:, i, :],
                    rhs=U_sb,
                    start=True,
                    stop=False,
                )
                pe.matmul(
                    ps_pre[:, sl, :],
                    lhsT=I_sb,
                    rhs=kv_sb[:, i, :],
                    start=False,
                    stop=False,
                )
                pe.matmul(
                    ps_g[:, 2 * i : 2 * i + 2],
                    lhsT=hT_sb[:, i, :],
                    rhs=sT_sb[:, t, 2 * i : 2 * i + 2],
                    start=True,
                    stop=True,
                )
                if BISECT == "nooneh":
                    pe.matmul(
                        ps_pre[:, sl, :],
                        lhsT=I_sb,
                        rhs=kv_sb[:, i, :],
                        start=False,
                        stop=True,
                    )
                else:
                    pe.matmul(
                        ps_pre[:, sl, :],
                        lhsT=oneh_sb[:, i, :],
                        rhs=swj_sb[:, t, :],
                        start=False,
                        stop=True,
                    )

            for h in range(2):  # half = groups 2h, 2h+1
                for g in range(2 * h, 2 * h + 2):
                    t0 = 4 * g
                    cols = slice(4 * g, 4 * g + 4)
                    # ---- gate: gamma = pad / (1 + exp(-(gdot + SK)))
                    if BISECT == "nostride":
                        act.activation(
                            glog_sb[:, cols], ps_g[:, 4 * g : 4 * g + 4], AF.Copy
                        )
                    elif BISECT == "noact":
                        vec.tensor_copy(
                            glog_sb[0:64, cols], ps_g[0:64, 8 * g : 8 * g + 8 : 2]
                        )
                        vec.tensor_copy(
                            glog_sb[64:128, cols],
                            ps_g[64:128, 8 * g + 1 : 8 * g + 8 : 2],
                        )
                    else:
                        act.activation(
                            glog_sb[0:64, cols],
                            ps_g[0:64, 8 * g : 8 * g + 8 : 2],
                            AF.Copy,
                        )
                        act.activation(
                            glog_sb[64:128, cols],
                            ps_g[64:128, 8 * g + 1 : 8 * g + 8 : 2],
                            AF.Copy,
                        )
                    vec.tensor_tensor(
                        glog_sb[:, cols], glog_sb[:, cols], SK_sb[:, t, cols],
                        op=ALU.add,
                    )
                    if BISECT == "noact":
                        vec.tensor_copy(ex_sb[:, cols], glog_sb[:, cols])
                    else:
                        act.activation(
                            ex_sb[:, cols], glog_sb[:, cols], AF.Exp, scale=-1.0
                        )
                    geng = pool if POOL_GATE else vec
                    geng.tensor_tensor(
                        den_sb[:, cols], ex_sb[:, cols], ones_sb[:, cols],
                        op=ALU.add,
                    )
                    vec.reciprocal(gam0_sb[:, cols], den_sb[:, cols])
                    geng.tensor_tensor(
                        gam_sb[:, cols], gam0_sb[:, cols], pad_sb[:, t, cols],
                        op=ALU.mult,
                    )

                    # ---- state update + row norms (n2 split DVE/ACT)
                    for i in range(t0, t0 + 4):
                        sl = i % NSLOT
                        vec._custom_dve(
                            op_upd,
                            out=upd_sb[:, i, :],
                            in0=upd_sb[:, i, :],
                            in1=ps_pre[:, sl, :],
                            s0=dl_sb[:, i : i + 1],
                            s1=gam_sb[:, i : i + 1],
                        )
                        if i < 10 or not ACT_TTR:
                            vec._custom_dve(
                                op_sq,
                                out=sq_sb[:, sl % 8, :],
                                in0=upd_sb[:, i, :],
                                s0=1e-12,
                                accum_out=n2_sb[:, i : i + 1],
                            )
                        else:
                            act.activation(
                                sq_sb[:, sl % 8, :],
                                upd_sb[:, i, :],
                                AF.Square,
                                accum_out=n2_sb[:, i : i + 1],
                            )

                # ---- delta = rsqrt(n2) for this half: exp-seed + 1 Newton
                hc = slice(8 * h, 8 * h + 8)
                vec.tensor_scalar_mul(n2h_sb[:, hc], n2_sb[:, hc], 0.5)
                if BISECT == "nobitcast":
                    vec.tensor_copy(Lf_sb[:, hc], n2_sb[:, hc])
                else:
                    vec.tensor_copy(
                        Lf_sb[:, hc], n2_sb[:, hc].bitcast(mybir.dt.int32)
                    )
                if BISECT == "noexpbias":
                    act.activation(
                        dl_sb[:, hc], Lf_sb[:, hc], AF.Exp,
                        scale=-4.1314791474339085e-08,
                    )
                elif BISECT == "noact":
                    vec.tensor_copy(dl_sb[:, hc], Lf_sb[:, hc])
                else:
                    act.activation(
                        dl_sb[:, hc], Lf_sb[:, hc], AF.Exp,
                        scale=-4.1314791474339085e-08, bias=sbias_sb[:],
                    )
                e0 = nwt_sb[:, 0, hc]
                e1 = nwt_sb[:, 1, hc]
                neng = pool if POOL_NEWTON else vec
                neng.tensor_tensor(e0, n2h_sb[:, hc], dl_sb[:, hc], op=ALU.mult)
                neng.tensor_tensor(e1, e0, dl_sb[:, hc], op=ALU.mult)
                neng.tensor_tensor(e0, c15_sb[:, hc], e1, op=ALU.subtract)
                neng.tensor_tensor(dl_sb[:, hc], dl_sb[:, hc], e0, op=ALU.mult)

                # ---- h = delta*upd (bf16) + per-group transposes
                for g in range(2 * h, 2 * h + 2):
                    t0 = 4 * g
                    for i in range(t0, t0 + 4):
                        if POOL_H:
                            pool.tensor_tensor(
                                h_sb[:, i, :],
                                upd_sb[:, i, :],
                                dl_sb[:, i : i + 1].broadcast_to([128, 128]),
                                op=ALU.mult,
                            )
                        else:
                            act.activation(
                                h_sb[:, i, :],
                                upd_sb[:, i, :],
                                AF.Copy,
                                scale=dl_sb[:, i : i + 1],
                            )
                    if BISECT == "notrans":
                        if g == NG - 1:
                            sync.dma_start_transpose(
                                hT_sb[:],
                                h_sb[:].rearrange("p a b -> p (a b)"),
                            )
                    else:
                        qeng = sync if (g % 2 == 0 or not ACTQ_TRANSPOSE) else act
                        qeng.dma_start_transpose(
                            hT_sb[:, t0 : t0 + 4, :],
                            h_sb[:, t0 : t0 + 4, :].rearrange("p a b -> p (a b)"),
                        )

        # ================= output =================
        for i in range(NT):
            act.activation(
                hfin_sb[:, i, :],
                upd_sb[:, i, :],
                AF.Copy,
                scale=dl_sb[:, i : i + 1],
            )
        sync.dma_start(hfin_out[:], hfin_sb[:])

    nc.compile()
    return nc


# ------------------------------------------------------------------ host prep
def prepare_inputs(encoded_sents, mask, keys, U, V, W):
    """Build per-core input maps + metadata. Returns (T, in_maps)."""
    es = np.asarray(encoded_sents, dtype=np.float32)
    mk = np.asarray(mask)
    ks = np.asarray(keys, dtype=np.float32)

    nb = mk.sum(axis=1).astype(np.int64)  # active counts per paragraph
    T = int(nb.max()) if nb.max() > 0 else 1

    bf = ml_dtypes.bfloat16
    U_b = np.asarray(U, dtype=np.float32).astype(bf)
    V_b = np.asarray(V, dtype=np.float32).astype(bf)
    W_b = np.asarray(W, dtype=np.float32).astype(bf)
    ident = np.eye(128, dtype=np.float32).astype(bf)

    # onehot[j, i*128+p] = 1 if paragraph_of(p, i) == j; b(p,i) = 2i + (p>=64)
    q = np.arange(128)
    i_idx = np.arange(NT)
    b_loc = 2 * i_idx[None, :] + (q[:, None] >= 64)  # [128, NT]
    oneh = np.zeros((32, NT, 128), np.float32)
    for i in range(NT):
        for p in range(128):
            oneh[b_loc[p, i], i, p] = 1.0
    oneh = oneh.reshape(32, NT * 128).astype(bf)

    in_maps = []
    for c in range(NCORES):
        bs = np.arange(BL) + BL * c  # global paragraph ids
        s_comp = np.zeros((BL, T, D), np.float32)
        padm = np.zeros((BL, T), np.float32)
        for j, b in enumerate(bs):
            idx = np.nonzero(mk[b])[0]
            n = len(idx)
            if n:
                s_comp[j, :n] = es[b, idx]
                padm[j, :n] = 1.0

        # sT[d, t, j]
        sT = np.ascontiguousarray(s_comp.transpose(2, 1, 0)).astype(bf)

        # keysT[d, i, q] = keys[b(i,q), k(q), d];  b_loc = 2i + (q>=64), k = q%64
        kk = ks[bs]  # [BL, K, D]
        k_of_q = q % 64
        keysT = np.ascontiguousarray(
            kk[b_loc, k_of_q[:, None], :].transpose(2, 1, 0)
        ).astype(bf)
        # keysT now [D, NT, 128]

        # pad[p, t, i] = padm[b_loc(p, i), t]
        padf = np.ascontiguousarray(
            padm[b_loc, :].transpose(0, 2, 1)
        ).astype(np.float32)
        # padf [128, T, NT]

        blob = np.concatenate(
            [
                keysT.reshape(D, NT * 128),
                sT.reshape(D, T * BL),
                U_b,
                V_b,
                W_b,
                ident,
            ],
            axis=1,
        ).astype(bf)
        in_maps.append(
            {"blob16": np.ascontiguousarray(blob), "oneh": oneh, "pad": padf}
        )
    return T, in_maps


def gather_output(results):
    """results: list of dicts with 'hfin' [128, NT, 128] per core -> [B, K, D]."""
    out = np.zeros((B, K, D), np.float32)
    for c in range(NCORES):
        h = results[c]["hfin"]  # [128, NT, 128]
        for b_loc in range(BL):
            i, a = b_loc // 2, b_loc % 2
            out[BL * c + b_loc] = h[64 * a : 64 * a + 64, i, :]
    return out


# ------------------------------------------------------------------ entry
def kernel(encoded_sents, mask, keys, U, V, W):
    from concourse.bass_utils import run_bass_kernel_spmd

    T, in_maps = prepare_inputs(encoded_sents, mask, keys, U, V, W)
    nc = build_program(T)
    res = run_bass_kernel_spmd(nc, in_maps, core_ids=list(range(NCORES)))
    return gather_output(res.results)


# ------------------------------------------------------------------ sim check
def _sim_check():
    """CoreSim single-core run on truncated data vs numpy emulation."""
    from concourse import bass_interp
    import jax

    sys.path.insert(0, os.path.dirname(os.path.abspath(__file__)))
    import reference

    inputs = {k: np.asarray(v) for k, v in reference.setup_inputs().items()}
    # truncate so the sim is fast: keep only first 6 active steps per paragraph
    mask = inputs["mask"].copy()
    for b in range(B):
        idx = np.nonzero(mask[b])[0]
        mask[b, idx[6:]] = False
    inputs["mask"] = mask

    ref = np.asarray(
        reference.reference(
            inputs["encoded_sents"],
            mask,
            inputs["keys"],
            inputs["U"],
            inputs["V"],
            inputs["W"],
        )
    )

    T, in_maps = prepare_inputs(
        inputs["encoded_sents"], mask, inputs["keys"],
        inputs["U"], inputs["V"], inputs["W"],
    )
    print(f"sim T={T}")
    nc = build_program(T, sim=True)
    core = 0
    sim = bass_interp.CoreSim(nc)
    for k, v in in_maps[core].items():
        sim.tensor(k)[:] = v
    sim.simulate()
    got = gather_output([{"hfin": np.array(sim.tensor("hfin"))}] * NCORES)

    g0 = got[:BL]
    r0 = ref[:BL]
    denom = np.abs(r0).max()
    err = np.abs(g0 - r0).max() / denom
    rel = np.linalg.norm(g0 - r0) / np.linalg.norm(r0)
    print(f"sim core0: absmax-rel {err:.3e}  l2-rel {rel:.3e}")
    return err


if __name__ == "__main__":
    _sim_check()


# revision 40
# speedup vs baseline: 1.1036x; 1.1036x over previous
"""Trainium2 Bass kernel for nn_BasicRecurrentEntityEncoder (v2).

Math (per paragraph b, per step t, state h [K, D]):
    g   = sigmoid(s . (h + keys))            per entity gate
    ht  = relu(h U + keys V + s W)
    upd = h + g * ht
    h'  = upd / ||upd||_2   where active (mask), else h unchanged

Device mapping (8 cores, pure data parallel, 32 paragraphs/core):
  * rows r = (paragraph, entity) = 2048 rows/core, 16 tiles of 128.
  * masked steps are identity -> host compresses each paragraph's sequence
    to its active steps; loop runs T = max active count steps with a
    per-(row, step) pad mask zeroing the gate on the tail (normalize is
    scale-invariant so no pad term is needed in delta).
  * state: upd (fp32, unnormalized), dl (per-row 1/norm), h = dl*upd
    (bf16) + hT (bf16 transposed shadow via per-group DMA transposes on
    the two HWDGE queues) feeding the PE.
  * per step, per tile i (PSUM slot i%8):
      U:  pre  = hT_i @ U        (stationary = hT tile, rhs U)      start
      kv: pre += I @ kv_i                                           accum
      g:  gdot = hT_i @ s cols   (2 cols -> ps_g)                   own group
      sw: pre += onehot_i @ sw_j[t]  (one-hot paragraph broadcast)  stop
  * gate per 4-tile group: ACT strided psum copies + SK add (DVE) ->
    ACT Sigmoid -> Pool mult by pad. Single ACT table set (sigmoid).
  * update per tile: custom DVE op upd' = upd*dl + relu(pre*gamma), then
    DVE tensor_tensor_reduce n2 = sum(upd'^2) + 1e-12.
  * delta per group: DVE tensor_scalar pow: dl = n2^-0.5.
  * h per tile on Pool: h = dl*upd (bf16); per-group DMA transpose to hT.
"""

import os
import sys

sys.path.insert(0, "/opt/trn_rl_repo")

import numpy as np
import ml_dtypes
from contextlib import ExitStack

import concourse.bass as bass
import concourse.bacc as bacc
import concourse.mybir as mybir
from concourse.tile import TileContext

BF16 = mybir.dt.bfloat16
F32 = mybir.dt.float32
AF = mybir.ActivationFunctionType
ALU = mybir.AluOpType

B, S, K, D = 256, 256, 64, 128
NCORES = 8
BL = B // NCORES  # 32 local paragraphs
NT = 16  # row tiles per core (2048 rows / 128)
NG = 4  # tile groups (4 tiles each)
NSLOT = 16  # psum slots for pre (one per tile: WAR edges stay in emission order)

# engine-placement knobs (bisectable: conservative = False everywhere)
POOL_GATE = True  # den/gam on Pool vs DVE
POOL_NEWTON = True  # newton iterations on Pool vs DVE
ACT_TTR = True  # 6 of 16 n2 reductions on ACT Square+accum vs all-DVE sq-op
ACTQ_TRANSPOSE = True  # alternate transposes across qSP/qACT vs all qSP
POOL_H_TILES = {0, 1, 2, 8, 9, 10}  # h-copies on Pool; rest on ACT
ACT_SQ_TILES = {5, 6, 7, 13, 14, 15}  # n2 via ACT Square+accum; rest DVE sq-op
BISECT = "full"  # crash-bisect variants; see bisect.py


# ------------------------------------------------------------------ custom op
def get_update_op():
    """out = in0*s0 + relu(in1*s1)   (upd*delta + relu(pre*gamma))."""
    from concourse import dve_ops as dv
    from concourse.dve_spec import Spec, Src0, Src1, C0, C1, relu, lower, _has_src1
    from concourse.dve_uop import DveOpSpec

    name = "SCALE_ADD_RELU_SCALED_ANT"
    for o in dv.OPS:
        if o.name == name:
            return o

    def _ref(in0, in1, s0, s1, imm2):
        x = in1.astype(np.float32) * s1
        x = np.nan_to_num(x, nan=0.0, posinf=np.inf, neginf=-np.inf)
        return in0.astype(np.float32) * s0 + np.maximum(x, 0.0)

    spec = Spec(body=Src0 * C0 + relu(Src1 * C1), reference=_ref)
    return _register_dve_op(name, spec)


def _register_dve_op(name, spec):
    from concourse import dve_ops as dv
    from concourse.dve_spec import lower, _has_src1
    from concourse.dve_uop import DveOpSpec

    row = max(dv._SUB_OPCODE_FOR_NAME.values()) + 1
    assert row < 0x20, "no free custom-DVE opcode rows"
    dv._SUB_OPCODE_FOR_NAME[name] = row
    shas = {}
    for ver in ("v3", "v4"):
        try:
            uops = lower(spec, ver=ver)
            shas[ver] = DveOpSpec(
                name=name, opcode=row, uops=uops, rd1_en=_has_src1(spec)
            ).sha(ver)
        except Exception:
            pass
    assert "v3" in shas, "custom op failed to lower for TRN2"
    op = dv.DveOp(name, spec, subdim=False, uops_sha=shas)
    dv.OPS.append(op)
    dv.CUSTOM_DVE_SPECS[name] = spec
    return op


def get_sq_accum_op():
    """out = in0^2 ; accum_out = s0 + sum(out)   (row sq-norm for delta)."""
    from concourse import dve_ops as dv
    from concourse.dve_spec import Spec, Src0, C0, sq
    from operator import add

    name = "SQ_ACCUM_ANT"
    for o in dv.OPS:
        if o.name == name:
            return o

    def _ref(in0, in1, s0, s1, imm2):
        b = (in0.astype(np.float32) ** 2).astype(np.float32)
        return b, s0 + b.reshape(b.shape[0], -1).sum(axis=-1, keepdims=True)

    spec = Spec(body=sq(Src0), accum=add, accum_init=C0, reference=_ref)
    return _register_dve_op(name, spec)


# ------------------------------------------------------------------ program
def build_program(T, sim=False):
    """Emit the full per-core Bass program. Returns nc."""
    op_upd = get_update_op()
    op_sq = get_sq_accum_op()
    nc = bacc.Bacc("TRN2", target_bir_lowering=False)

    # ---- I/O (packed blobs: few DMA queues -> few sem waits downstream)
    NFB = NT * 128 + T * BL + 4 * 128  # keysT | sT | U | V | W | ident
    blob_in = nc.dram_tensor("blob16", [128, NFB], BF16, kind="ExternalInput")
    oneh_in = nc.dram_tensor("oneh", [32, NT * 128], BF16, kind="ExternalInput")
    pad_in = nc.dram_tensor("pad", [128, T, NT], F32, kind="ExternalInput")
    hfin_out = nc.dram_tensor("hfin", [128, NT, 128], F32, kind="ExternalOutput")

    with ExitStack() as ctx:
        tc = ctx.enter_context(TileContext(nc))
        ec = ctx.enter_context

        # ---- persistent SBUF
        blob_sb = ec(nc.sbuf_tensor("blob_sb", [128, NFB], BF16))
        o = 0
        keysT_sb = blob_sb[:, o : o + NT * 128].rearrange(
            "p (i e) -> p i e", i=NT
        ); o += NT * 128
        sT_sb = blob_sb[:, o : o + T * BL].rearrange(
            "p (t j) -> p t j", t=T
        ); o += T * BL
        U_sb = blob_sb[:, o : o + 128]; o += 128
        V_sb = blob_sb[:, o : o + 128]; o += 128
        W_sb = blob_sb[:, o : o + 128]; o += 128
        I_sb = blob_sb[:, o : o + 128]; o += 128
        oneh_sb = ec(nc.sbuf_tensor("oneh_sb", [32, NT, 128], BF16))
        pad_sb = ec(nc.sbuf_tensor("pad_sb", [128, T, NT], F32))
        kv_sb = ec(nc.sbuf_tensor("kv_sb", [128, NT, 128], BF16))
        SK_sb = ec(nc.sbuf_tensor("SK_sb", [128, T, NT], F32))
        SK2_sb = ec(nc.sbuf_tensor("SK2_sb", [128, T, 2 * NT], BF16))
        swj_sb = ec(nc.sbuf_tensor("swj_sb", [32, T, 128], BF16))
        h_sb = ec(nc.sbuf_tensor("h_sb", [128, NT, 128], BF16))
        hT_sb = ec(nc.sbuf_tensor("hT_sb", [128, NT, 128], BF16))
        upd_sb = ec(nc.sbuf_tensor("upd_sb", [128, NT, 128], F32))
        sq_sb = ec(nc.sbuf_tensor("sq_sb", [128, 8, 128], F32))
        dl_sb = ec(nc.sbuf_tensor("dl_sb", [128, NT], F32))
        n2_sb = ec(nc.sbuf_tensor("n2_sb", [128, NT], F32))
        glog_sb = ec(nc.sbuf_tensor("glog_sb", [128, NT], F32))
        ex_sb = ec(nc.sbuf_tensor("ex_sb", [128, NT], F32))
        den_sb = ec(nc.sbuf_tensor("den_sb", [128, NT], F32))
        gam0_sb = ec(nc.sbuf_tensor("gam0_sb", [128, NT], F32))
        gam_sb = ec(nc.sbuf_tensor("gam_sb", [128, NT], F32))
        ones_sb = ec(nc.sbuf_tensor("ones_sb", [128, NT], F32))
        c15_sb = ec(nc.sbuf_tensor("c15_sb", [128, NT], F32))
        sbias_sb = ec(nc.sbuf_tensor("sbias_sb", [128, 1], F32))
        n2h_sb = ec(nc.sbuf_tensor("n2h_sb", [128, NT], F32))
        Lf_sb = ec(nc.sbuf_tensor("Lf_sb", [128, NT], F32))
        nwt_sb = ec(nc.sbuf_tensor("nwt_sb", [128, 2, NT], F32))
        hfin_sb = ec(nc.sbuf_tensor("hfin_sb", [128, NT, 128], F32))
        # psum
        ps_pre = ec(nc.psum_tensor("ps_pre", [128, NSLOT, 128], F32))
        ps_g = ec(nc.psum_tensor("ps_g", [128, 2 * NT], F32))
        ps_aux = ec(nc.psum_tensor("ps_aux", [128, 512], F32))

        sync = nc.sync
        vec = nc.vector
        act = nc.scalar
        pool = nc.gpsimd
        pe = nc.tensor

        # ================= setup =================
        sync.dma_start(blob_sb[:], blob_in[:], max_dma_last_dim=65024)
        sync.dma_start(pad_sb[:], pad_in[:], max_dma_last_dim=65024)
        sync.dma_start(
            oneh_sb[:].rearrange("p a b -> p (a b)"), oneh_in[:],
            max_dma_last_dim=65024,
        )

        vec.memset(h_sb[:], 0)
        vec.memset(hT_sb[:], 0)
        vec.memset(upd_sb[:], 0.0)
        vec.memset(dl_sb[:], 1.0)
        vec.memset(ones_sb[:], 1.0)
        vec.memset(c15_sb[:], 1.5)
        vec.memset(sbias_sb[:], 43.9998)

        # kv = keys @ V   (natural tiles), 4 tiles per psum round
        for c in range(4):
            for k in range(4):
                i = 4 * c + k
                pe.matmul(
                    ps_aux[:, k * 128 : (k + 1) * 128],
                    lhsT=keysT_sb[:, i, :],
                    rhs=V_sb,
                    start=True,
                    stop=True,
                )
            vec.tensor_copy(
                kv_sb[:, 4 * c : 4 * (c + 1), :], ps_aux[:]
            )

        # SK[r, t] = s_{b(r), t} . keys_r
        for i in range(NT):
            for j in range(2):
                pe.matmul(
                    ps_aux[:, 0:T],
                    lhsT=keysT_sb[:, i, :],
                    rhs=sT_sb[:, :, 2 * i + j],
                    start=True,
                    stop=True,
                )
                half = slice(0, 64) if j == 0 else slice(64, 128)
                vec.tensor_copy(SK_sb[half, :, i], ps_aux[half, 0:T])
        # SK2[p, t, 2i+a] = SK[p, t, i] (bf16, rides the ps_g accumulation)
        vec.tensor_copy(
            SK2_sb[:].rearrange("p t (i a) -> p t i a", a=2),
            SK_sb[:].broadcast_to([128, T, NT, 2]),
        )

        # sw_j[j, t, e] = (s_{t,j} @ W)[e], 4 steps per psum round
        for tc0 in range(0, T, 4):
            n = min(4, T - tc0)
            for k in range(n):
                pe.matmul(
                    ps_aux[0:32, k * 128 : (k + 1) * 128],
                    lhsT=sT_sb[:, tc0 + k, :],
                    rhs=W_sb,
                    start=True,
                    stop=True,
                )
            vec.tensor_copy(
                swj_sb[:, tc0 : tc0 + n, :], ps_aux[0:32, 0 : n * 128]
            )

        # ================= time loop =================
        for t in range(T):
            # ---- PE: per tile [U, kv, gate, SK-add, sw]; 2-col matmuls ride
            # between 128-col matmuls so most ldweights stay hidden. ps_g
            # accumulates gdot + SK so ACT can exp() it straight from psum.
            for i in range(NT):
                sl = i % NSLOT
                pe.matmul(
                    ps_pre[:, sl, :],
                    lhsT=hT_sb[:, i, :],
                    rhs=U_sb,
                    start=True,
                    stop=False,
                )
                pe.matmul(
                    ps_pre[:, sl, :],
                    lhsT=I_sb,
                    rhs=kv_sb[:, i, :],
                    start=False,
                    stop=False,
                )
                pe.matmul(
                    ps_g[:, 2 * i : 2 * i + 2],
                    lhsT=hT_sb[:, i, :],
                    rhs=sT_sb[:, t, 2 * i : 2 * i + 2],
                    start=True,
                    stop=False,
                )
                pe.matmul(
                    ps_g[:, 2 * i : 2 * i + 2],
                    lhsT=I_sb,
                    rhs=SK2_sb[:, t, 2 * i : 2 * i + 2],
                    start=False,
                    stop=True,
                )
                pe.matmul(
                    ps_pre[:, sl, :],
                    lhsT=oneh_sb[:, i, :],
                    rhs=swj_sb[:, t, :],
                    start=False,
                    stop=True,
                )

            for h in range(2):  # half = groups 2h, 2h+1
                for g in range(2 * h, 2 * h + 2):
                    t0 = 4 * g
                    cols = slice(4 * g, 4 * g + 4)
                    # ---- gate: gamma = pad / (1 + exp(-(gdot + SK)))
                    act.activation(
                        ex_sb[0:64, cols],
                        ps_g[0:64, 8 * g : 8 * g + 8 : 2],
                        AF.Exp,
                        scale=-1.0,
                    )
                    act.activation(
                        ex_sb[64:128, cols],
                        ps_g[64:128, 8 * g + 1 : 8 * g + 8 : 2],
                        AF.Exp,
                        scale=-1.0,
                    )
                    geng = pool if POOL_GATE else vec
                    geng.tensor_tensor(
                        den_sb[:, cols], ex_sb[:, cols], ones_sb[:, cols],
                        op=ALU.add,
                    )
                    vec.reciprocal(gam0_sb[:, cols], den_sb[:, cols])
                    geng.tensor_tensor(
                        gam_sb[:, cols], gam0_sb[:, cols], pad_sb[:, t, cols],
                        op=ALU.mult,
                    )

                    # ---- state update + row norms (n2 split DVE/ACT)
                    for i in range(t0, t0 + 4):
                        sl = i % NSLOT
                        vec._custom_dve(
                            op_upd,
                            out=upd_sb[:, i, :],
                            in0=upd_sb[:, i, :],
                            in1=ps_pre[:, sl, :],
                            s0=dl_sb[:, i : i + 1],
                            s1=gam_sb[:, i : i + 1],
                        )
                        if not ACT_TTR or i not in ACT_SQ_TILES:
                            vec._custom_dve(
                                op_sq,
                                out=sq_sb[:, sl % 8, :],
                                in0=upd_sb[:, i, :],
                                s0=1e-12,
                                accum_out=n2_sb[:, i : i + 1],
                            )
                        else:
                            act.activation(
                                sq_sb[:, sl % 8, :],
                                upd_sb[:, i, :],
                                AF.Square,
                                accum_out=n2_sb[:, i : i + 1],
                            )

                # ---- delta = rsqrt(n2) for this half: exp-seed + 1 Newton
                hc = slice(8 * h, 8 * h + 8)
                vec.tensor_scalar_mul(n2h_sb[:, hc], n2_sb[:, hc], 0.5)
                vec.tensor_copy(
                    Lf_sb[:, hc], n2_sb[:, hc].bitcast(mybir.dt.int32)
                )
                act.activation(
                    dl_sb[:, hc], Lf_sb[:, hc], AF.Exp,
                    scale=-4.1314791474339085e-08, bias=sbias_sb[:],
                )
                e0 = nwt_sb[:, 0, hc]
                e1 = nwt_sb[:, 1, hc]
                neng = pool if POOL_NEWTON else vec
                neng.tensor_tensor(e0, n2h_sb[:, hc], dl_sb[:, hc], op=ALU.mult)
                neng.tensor_tensor(e1, e0, dl_sb[:, hc], op=ALU.mult)
                neng.tensor_tensor(e0, c15_sb[:, hc], e1, op=ALU.subtract)
                neng.tensor_tensor(dl_sb[:, hc], dl_sb[:, hc], e0, op=ALU.mult)

                # ---- h = delta*upd (bf16) + per-group transposes
                for g in range(2 * h, 2 * h + 2):
                    t0 = 4 * g
                    for i in range(t0, t0 + 4):
                        if i in POOL_H_TILES:
                            pool.tensor_tensor(
                                h_sb[:, i, :],
                                upd_sb[:, i, :],
                                dl_sb[:, i : i + 1].broadcast_to([128, 128]),
                                op=ALU.mult,
                            )
                        else:
                            act.activation(
                                h_sb[:, i, :],
                                upd_sb[:, i, :],
                                AF.Copy,
                                scale=dl_sb[:, i : i + 1],
                            )
                    qeng = sync if (g % 2 == 0 or not ACTQ_TRANSPOSE) else act
                    qeng.dma_start_transpose(
                        hT_sb[:, t0 : t0 + 4, :],
                        h_sb[:, t0 : t0 + 4, :].rearrange("p a b -> p (a b)"),
                    )

        # ================= output =================
        for i in range(NT):
            act.activation(
                hfin_sb[:, i, :],
                upd_sb[:, i, :],
                AF.Copy,
                scale=dl_sb[:, i : i + 1],
            )
        sync.dma_start(hfin_out[:], hfin_sb[:])

    nc.compile()
    return nc


# ------------------------------------------------------------------ host prep
def prepare_inputs(encoded_sents, mask, keys, U, V, W):
    """Build per-core input maps + metadata. Returns (T, in_maps)."""
    es = np.asarray(encoded_sents, dtype=np.float32)
    mk = np.asarray(mask)
    ks = np.asarray(keys, dtype=np.float32)

    nb = mk.sum(axis=1).astype(np.int64)  # active counts per paragraph
    T = int(nb.max()) if nb.max() > 0 else 1

    bf = ml_dtypes.bfloat16
    U_b = np.asarray(U, dtype=np.float32).astype(bf)
    V_b = np.asarray(V, dtype=np.float32).astype(bf)
    W_b = np.asarray(W, dtype=np.float32).astype(bf)
    ident = np.eye(128, dtype=np.float32).astype(bf)

    # onehot[j, i*128+p] = 1 if paragraph_of(p, i) == j; b(p,i) = 2i + (p>=64)
    q = np.arange(128)
    i_idx = np.arange(NT)
    b_loc = 2 * i_idx[None, :] + (q[:, None] >= 64)  # [128, NT]
    oneh = np.zeros((32, NT, 128), np.float32)
    for i in range(NT):
        for p in range(128):
            oneh[b_loc[p, i], i, p] = 1.0
    oneh = oneh.reshape(32, NT * 128).astype(bf)

    in_maps = []
    for c in range(NCORES):
        bs = np.arange(BL) + BL * c  # global paragraph ids
        s_comp = np.zeros((BL, T, D), np.float32)
        padm = np.zeros((BL, T), np.float32)
        for j, b in enumerate(bs):
            idx = np.nonzero(mk[b])[0]
            n = len(idx)
            if n:
                s_comp[j, :n] = es[b, idx]
                padm[j, :n] = 1.0

        # sT[d, t, j]
        sT = np.ascontiguousarray(s_comp.transpose(2, 1, 0)).astype(bf)

        # keysT[d, i, q] = keys[b(i,q), k(q), d];  b_loc = 2i + (q>=64), k = q%64
        kk = ks[bs]  # [BL, K, D]
        k_of_q = q % 64
        keysT = np.ascontiguousarray(
            kk[b_loc, k_of_q[:, None], :].transpose(2, 1, 0)
        ).astype(bf)
        # keysT now [D, NT, 128]

        # pad[p, t, i] = padm[b_loc(p, i), t]
        padf = np.ascontiguousarray(
            padm[b_loc, :].transpose(0, 2, 1)
        ).astype(np.float32)
        # padf [128, T, NT]

        blob = np.concatenate(
            [
                keysT.reshape(D, NT * 128),
                sT.reshape(D, T * BL),
                U_b,
                V_b,
                W_b,
                ident,
            ],
            axis=1,
        ).astype(bf)
        in_maps.append(
            {"blob16": np.ascontiguousarray(blob), "oneh": oneh, "pad": padf}
        )
    return T, in_maps


def gather_output(results):
    """results: list of dicts with 'hfin' [128, NT, 128] per core -> [B, K, D]."""
    out = np.zeros((B, K, D), np.float32)
    for c in range(NCORES):
        h = results[c]["hfin"]  # [128, NT, 128]
        for b_loc in range(BL):
            i, a = b_loc // 2, b_loc % 2
            out[BL * c + b_loc] = h[64 * a : 64 * a + 64, i, :]
    return out


# ------------------------------------------------------------------ entry
def kernel(encoded_sents, mask, keys, U, V, W):
    from concourse.bass_utils import run_bass_kernel_spmd

    T, in_maps = prepare_inputs(encoded_sents, mask, keys, U, V, W)
    nc = build_program(T)
    res = run_bass_kernel_spmd(nc, in_maps, core_ids=list(range(NCORES)))
    return gather_output(res.results)


# ------------------------------------------------------------------ sim check
def _sim_check():
    """CoreSim single-core run on truncated data vs numpy emulation."""
    from concourse import bass_interp
    import jax

    sys.path.insert(0, os.path.dirname(os.path.abspath(__file__)))
    import reference

    inputs = {k: np.asarray(v) for k, v in reference.setup_inputs().items()}
    # truncate so the sim is fast: keep only first 6 active steps per paragraph
    mask = inputs["mask"].copy()
    for b in range(B):
        idx = np.nonzero(mask[b])[0]
        mask[b, idx[6:]] = False
    inputs["mask"] = mask

    ref = np.asarray(
        reference.reference(
            inputs["encoded_sents"],
            mask,
            inputs["keys"],
            inputs["U"],
            inputs["V"],
            inputs["W"],
        )
    )

    T, in_maps = prepare_inputs(
        inputs["encoded_sents"], mask, inputs["keys"],
        inputs["U"], inputs["V"], inputs["W"],
    )
    print(f"sim T={T}")
    nc = build_program(T, sim=True)
    core = 0
    sim = bass_interp.CoreSim(nc)
    for k, v in in_maps[core].items():
        sim.tensor(k)[:] = v
    sim.simulate()
    got = gather_output([{"hfin": np.array(sim.tensor("hfin"))}] * NCORES)

    g0 = got[:BL]
    r0 = ref[:BL]
    denom = np.abs(r0).max()
    err = np.abs(g0 - r0).max() / denom
    rel = np.linalg.norm(g0 - r0) / np.linalg.norm(r0)
    print(f"sim core0: absmax-rel {err:.3e}  l2-rel {rel:.3e}")
    return err


if __name__ == "__main__":
    _sim_check()
